# revision 8
# baseline (speedup 1.0000x reference)
"""GAT edge-softmax aggregation kernel for 8 Trainium2 NeuronCores.

Strategy (edge-parallel, dst-range sharded):
  - Edges are partitioned by dst//6250 so each core owns all edges of a
    6250-node dst range: segment reductions become core-local (no collectives).
  - Per core: dense precompute a = x@w1[:64]+b1, b = x@w1[64:] as fp16 tables
    in DRAM; per-edge h^T columns arrive via SWDGE dma_gather (transposed,
    256B rows); logits via per-tile PE matmul with w2; e = exp(logits) on ACT
    (softmax max-subtraction skipped -- shift-invariant and |logit| < 10).
  - Segment sums via one-hot matmuls accumulated in PSUM: dst = hi*128+lo,
    den[lo,hi] += sum_p onehot_lo(p) * e_p*onehot_hi(p), same for y*e.
  - alpha = e * (1/den)[dst] via a second 256B-row gather from a replicated
    reciprocal table; y_hat = ynum*recip(den) per node.
"""
import numpy as np

N = 50000
E = 1_600_000
W = 64
H = 128
NCORES = 8
RANGE = 6250
P = 128
HI = 49
NBINS = P * HI          # 6272 (6250 real + 22 pad bins)
EC = 204_800            # padded edges per core
C = EC // P             # 1600 columns (partition-major layout)
CH = 64                 # chunk = CH*128 = 8192 edges
NCH = C // CH           # 25 chunks
REBASE = 17_232         # int16 table rebase: idx = global - REBASE in [-17232, 32767]
NPAD = 50_048           # table rows (50000 padded to 128 mult)



# ---- inlined walrus workaround (this build allows only ONE sync-wait per
# instruction): split extra waits onto single-wait NoOps, hooked into
# Bacc.compile so it runs before freeze ----
_PATCH = {"done": False}


def _split_waits(nc):
    import concourse.mybir as mybir
    ctr = 0
    for f in nc.m.functions:
        for blk in f.blocks:
            il = blk.instructions
            i = 0
            while i < len(il):
                inst = il[i]
                si = inst.sync_info
                if si is None or not si.on_wait or len(si.on_wait) <= 1:
                    i += 1
                    continue
                waits = list(si.on_wait)
                si.on_wait = waits[-1:]
                for w in waits[:-1]:
                    ctr += 1
                    n = mybir.InstNoOp(name=f"waitsplit-{ctr}", ins=[], outs=[])
                    n.engine = inst.engine
                    n.sync_info = mybir.SyncInfo(on_wait=[w], on_update=[])
                    il.insert(i, n)
                    i += 1
                i += 1


def _apply_bacc_patch():
    if _PATCH["done"]:
        return
    from concourse.bacc import Bacc
    orig = Bacc.compile

    def patched(self):
        orig(self)
        _split_waits(self)

    Bacc.compile = patched
    _PATCH["done"] = True


def _build():
    _apply_bacc_patch()
    import concourse.bass as bass
    import concourse.mybir as mybir
    from concourse.bacc import Bacc
    from concourse.tile import TileContext
    from concourse.tile_rust import add_dep_helper

    dt = mybir.dt
    F16 = dt.float16
    F32 = dt.float32
    ALU = mybir.AluOpType
    ACTF = mybir.ActivationFunctionType

    nc = Bacc(trn_type="TRN2")

    # inputs
    xT1 = nc.dram_tensor("xT1", [W + 1, NPAD], F16, kind="ExternalInput")
    xT1b = nc.dram_tensor("xT1b", [W + 1, NPAD], F16, kind="ExternalInput")
    w1catb = nc.dram_tensor("w1catb", [W + 1, 2 * H], F16, kind="ExternalInput")
    y_pad = nc.dram_tensor("y_pad", [NPAD], F32, kind="ExternalInput")
    w2c = nc.dram_tensor("w2c", [H, 1], F16, kind="ExternalInput")
    b2c = nc.dram_tensor("b2c", [P, 1], F32, kind="ExternalInput")
    srci16 = nc.dram_tensor("srci16", [P, EC // 16], dt.int16, kind="ExternalInput")
    dloci16 = nc.dram_tensor("dloci16", [P, EC // 16], dt.int16, kind="ExternalInput")
    lo16 = nc.dram_tensor("lo16", [P, C], F16, kind="ExternalInput")
    hi16 = nc.dram_tensor("hi16", [P, C], F16, kind="ExternalInput")

    # internal DRAM scratch
    atab = nc.dram_tensor("atab", [NPAD, H], F16, kind="Internal")
    btab = nc.dram_tensor("btab", [NPAD, H], F16, kind="Internal")
    y64 = nc.dram_tensor("y64", [NPAD, 64], F32, kind="Internal")
    r64 = nc.dram_tensor("r64", [NBINS, 64], F32, kind="Internal")

    # outputs
    alpha_o = nc.dram_tensor("alpha_o", [P, C], F32, kind="ExternalOutput")
    yhat_o = nc.dram_tensor("yhat_o", [P, HI], F32, kind="ExternalOutput")
    den_o = nc.dram_tensor("den_o", [P, HI], F32, kind="ExternalOutput")

    NT = NPAD // P  # 391 node tiles

    with TileContext(nc) as tc:
        with (
            tc.tile_pool(name="const", bufs=1) as cpool,
            tc.tile_pool(name="stagea", bufs=3) as apool,
            tc.tile_pool(name="apsum", bufs=2, space="PSUM") as apsum,
            tc.tile_pool(name="big", bufs=2) as bigp,
            tc.tile_pool(name="med", bufs=2) as medp,
            tc.tile_pool(name="g64", bufs=1) as g64p,
            tc.tile_pool(name="yexpp", bufs=1) as yexpp,
            tc.tile_pool(name="small", bufs=3) as smp,
            tc.tile_pool(name="psum", bufs=2, space="PSUM") as psum,
            tc.tile_pool(name="hpsum", bufs=1, space="PSUM") as hpsum,
        ):
            table_writes = []
            gathers_ab = []

            # ---- constants in SBUF
            w1c_sb = cpool.tile([W + 1, 2 * H], F16)
            nc.sync.dma_start(out=w1c_sb[:], in_=w1catb[:])
            w2_sb = cpool.tile([H, 1], F16)
            nc.sync.dma_start(out=w2_sb[:], in_=w2c[:])
            b2_sb = cpool.tile([P, 1], F32)
            nc.sync.dma_start(out=b2_sb[:], in_=b2c[:])
            iotaL = cpool.tile([P, 1, P], F16)   # iotaL[p, 0, j] = j
            iotaH = cpool.tile([P, 1, HI], F16)  # iotaH[p, 0, c] = c
            ioL_i = cpool.tile([P, P], dt.int32)
            ioH_i = cpool.tile([P, HI], dt.int32)
            nc.gpsimd.iota(ioL_i[:], pattern=[[1, P]], channel_multiplier=0)
            nc.gpsimd.iota(ioH_i[:], pattern=[[1, HI]], channel_multiplier=0)
            nc.vector.tensor_copy(out=iotaL[:, 0, :], in_=ioL_i[:])
            nc.vector.tensor_copy(out=iotaH[:, 0, :], in_=ioH_i[:])
            e_sb = cpool.tile([P, C], F32)
            y_sb = cpool.tile([P, NT], F32)
            nc.sync.dma_start(out=y_sb[:], in_=y_pad[:].rearrange("(p c) -> p c", p=P))

            # ---- stage A: a/b tables ----
            for j in range(NT):
                xt = apool.tile([W + 1, P], F16, tag="xt")
                nc.sync.dma_start(out=xt[:], in_=xT1[:, j * P:(j + 1) * P])
                ab_ps = apsum.tile([P, 2 * H], F32, space="PSUM", tag="abps")
                nc.tensor.matmul(out=ab_ps[:], lhsT=xt[:], rhs=w1c_sb[:],
                                 start=True, stop=True)
                ab16 = apool.tile([P, 2 * H], F16, tag="ab16")
                nc.scalar.copy(out=ab16[:], in_=ab_ps[:])
                wa = nc.sync.dma_start(out=atab[j * P:(j + 1) * P, :], in_=ab16[:, 0:H])
                table_writes.append(wa)

            # ---- stage A1b: b-table in dst-local order (rotated input) ----
            for j in range(50):
                xtb = apool.tile([W + 1, P], F16, tag="xt")
                nc.sync.dma_start(out=xtb[:], in_=xT1b[:, j * P:(j + 1) * P])
                ab_ps2 = apsum.tile([P, 2 * H], F32, space="PSUM", tag="abps")
                nc.tensor.matmul(out=ab_ps2[:], lhsT=xtb[:], rhs=w1c_sb[:],
                                 start=True, stop=True)
                ab16b = apool.tile([P, 2 * H], F16, tag="ab16")
                nc.scalar.copy(out=ab16b[:], in_=ab_ps2[:])
                wb2 = nc.sync.dma_start(out=btab[j * P:(j + 1) * P, :],
                                        in_=ab16b[:, H:2 * H])
                table_writes.append(wb2)

            # ---- stage A2: y64 expansion ----
            YCH = 49
            for cc in range(0, NT, YCH):
                w_ = min(YCH, NT - cc)
                yexp = yexpp.tile([P, YCH, 64], F32, tag="yexp")
                nc.vector.tensor_copy(
                    out=yexp[:, :w_, :],
                    in_=y_sb[:, cc:cc + w_].to_broadcast([P, w_, 64]))
                wv = nc.sync.dma_start(
                    out=y64[:].rearrange("(p c) j -> p c j", p=P)[:, cc:cc + w_, :],
                    in_=yexp[:, :w_, :])
                table_writes.append(wv)

            # ---- stage B: main edge sweep ----
            den_ps = hpsum.tile([P, HI], F32, space="PSUM")
            ynum_ps = hpsum.tile([P, HI], F32, space="PSUM")
            for k in range(NCH):
                i0 = k * CH * P // 16     # idx-col offset (int16 wrapped)
                c0 = k * CH               # column offset (partition-major)
                ia = smp.tile([P, CH * 8], dt.int16, tag="ia")
                ib = smp.tile([P, CH * 8], dt.int16, tag="ib")
                nc.sync.dma_start(out=ia[:], in_=srci16[:, i0:i0 + CH * 8])
                nc.sync.dma_start(out=ib[:], in_=dloci16[:, i0:i0 + CH * 8])
                lo_t = smp.tile([P, CH], F16, tag="lo")
                hi_t = smp.tile([P, CH], F16, tag="hi")
                nc.sync.dma_start(out=lo_t[:], in_=lo16[:, c0:c0 + CH])
                nc.sync.dma_start(out=hi_t[:], in_=hi16[:, c0:c0 + CH])

                ga = bigp.tile([P, 1, CH * P], F16, tag="ga")
                gb = bigp.tile([P, 1, CH * P], F16, tag="gb")
                y64t = g64p.tile([P, CH, 64], F32, tag="g64")
                NS = CH * P // 512
                for s in range(NS):
                    isl = slice(s * 32, (s + 1) * 32)
                    g1 = nc.gpsimd.dma_gather(
                        out_ap=ga[:, :, s * 512:(s + 1) * 512], in_ap=atab[REBASE:, :],
                        idxs_ap=ia[:, isl],
                        num_idxs=512, num_idxs_reg=512, elem_size=H, transpose=True)
                    g2 = nc.gpsimd.dma_gather(
                        out_ap=gb[:, :, s * 512:(s + 1) * 512], in_ap=btab[:, :],
                        idxs_ap=ib[:, isl],
                        num_idxs=512, num_idxs_reg=512, elem_size=H, transpose=True)
                    g3 = nc.gpsimd.dma_gather(
                        out_ap=y64t[:, s * 4:(s + 1) * 4, :], in_ap=y64[REBASE:, :],
                        idxs_ap=ia[:, isl],
                        num_idxs=512, num_idxs_reg=512, elem_size=64)
                    gathers_ab += [g1, g2, g3]

                # h = relu(a + b)
                nc.vector.tensor_tensor(out=ga[:], in0=ga[:], in1=gb[:], op=ALU.add)
                nc.scalar.activation(out=ga[:], in_=ga[:], func=ACTF.Relu)

                # logits -> e (partition-major)
                lps = psum.tile([P, CH], F32, space="PSUM", tag="lps")
                for t in range(CH):
                    nc.tensor.matmul(out=lps[:, t:t + 1],
                                     lhsT=ga[:, 0, t * P:(t + 1) * P],
                                     rhs=w2_sb[:], start=True, stop=True)
                nc.scalar.activation(out=e_sb[:, c0:c0 + CH], in_=lps[:],
                                     func=ACTF.Exp, bias=b2_sb[:], scale=1.0)

                e16 = smp.tile([P, CH], F16, tag="e16")
                nc.vector.tensor_copy(out=e16[:], in_=e_sb[:, c0:c0 + CH])
                ye16 = smp.tile([P, CH], F16, tag="ye16")
                nc.vector.tensor_tensor(out=ye16[:], in0=y64t[:, :, 0],
                                        in1=e_sb[:, c0:c0 + CH], op=ALU.mult)

                # one-hots
                Lt = bigp.tile([P, CH, P], F16, tag="L")
                nc.vector.tensor_tensor(
                    out=Lt[:], in0=lo_t[:].to_broadcast([P, CH, P]),
                    in1=iotaL[:].to_broadcast([P, CH, P]), op=ALU.is_equal)
                Ht = medp.tile([P, CH, HI], F16, tag="H")
                nc.vector.tensor_tensor(
                    out=Ht[:], in0=hi_t[:].to_broadcast([P, CH, HI]),
                    in1=iotaH[:].to_broadcast([P, CH, HI]), op=ALU.is_equal)
                VHe = medp.tile([P, CH, HI], F16, tag="VHe")
                nc.vector.tensor_tensor(out=VHe[:], in0=Ht[:],
                                        in1=e16[:].to_broadcast([P, CH, HI]), op=ALU.mult)
                VHy = medp.tile([P, CH, HI], F16, tag="VHy")
                nc.vector.tensor_tensor(out=VHy[:], in0=Ht[:],
                                        in1=ye16[:].to_broadcast([P, CH, HI]), op=ALU.mult)

                first = (k == 0)
                last = (k == NCH - 1)
                for t in range(CH):
                    nc.tensor.matmul(out=den_ps[:], lhsT=Lt[:, t, :], rhs=VHe[:, t, :],
                                     start=(first and t == 0), stop=(last and t == CH - 1))
                    nc.tensor.matmul(out=ynum_ps[:], lhsT=Lt[:, t, :], rhs=VHy[:, t, :],
                                     start=(first and t == 0), stop=(last and t == CH - 1))

            # ---- stage C: finalize grids ----
            den_sb = cpool.tile([P, HI], F32)
            ynum_sb = cpool.tile([P, HI], F32)
            nc.vector.tensor_copy(out=den_sb[:], in_=den_ps[:])
            nc.vector.tensor_copy(out=ynum_sb[:], in_=ynum_ps[:])
            nc.sync.dma_start(out=den_o[:], in_=den_sb[:])
            dguard = cpool.tile([P, HI], F32)
            nc.vector.tensor_scalar(out=dguard[:], in0=den_sb[:], scalar1=1e-30,
                                    scalar2=None, op0=ALU.max)
            r_sb = cpool.tile([P, HI], F32)
            nc.vector.reciprocal(out=r_sb[:], in_=dguard[:])
            yh = cpool.tile([P, HI], F32)
            nc.vector.tensor_tensor(out=yh[:], in0=ynum_sb[:], in1=r_sb[:], op=ALU.mult)
            nc.sync.dma_start(out=yhat_o[:], in_=yh[:])
            rexp = yexpp.tile([P, HI, 64], F32, tag="yexp")
            nc.vector.tensor_copy(out=rexp[:], in_=r_sb[:].to_broadcast([P, HI, 64]))
            wr = nc.sync.dma_start(
                out=r64[:].rearrange("(h l) j -> l h j", l=P), in_=rexp[:])

            # ---- stage D: alpha ----
            for k in range(NCH):
                i0 = k * CH * P // 16
                c0 = k * CH
                il = smp.tile([P, CH * 8], dt.int16, tag="il")
                nc.sync.dma_start(out=il[:], in_=dloci16[:, i0:i0 + CH * 8])
                r64t = g64p.tile([P, CH, 64], F32, tag="g64")
                for s in range(CH * P // 512):
                    g4 = nc.gpsimd.dma_gather(
                        out_ap=r64t[:, s * 4:(s + 1) * 4, :], in_ap=r64[:],
                        idxs_ap=il[:, s * 32:(s + 1) * 32],
                        num_idxs=512, num_idxs_reg=512, elem_size=64)
                    add_dep_helper(g4.ins, wr.ins, True, "r64 gather waits table write")
                al = smp.tile([P, CH], F32, tag="al")
                nc.vector.tensor_tensor(out=al[:], in0=e_sb[:, c0:c0 + CH],
                                        in1=r64t[:, :, 0], op=ALU.mult)
                nc.sync.dma_start(out=alpha_o[:, c0:c0 + CH], in_=al[:])

            # DRAM RAW deps: all table writes -> join -> all gathers
            join = nc.sync.nop(nofuse=True)
            for wv in table_writes:
                add_dep_helper(join.ins, wv.ins, True, "join waits table writes")
            for g in gathers_ab:
                add_dep_helper(g.ins, join.ins, True, "gathers wait join")

    nc.finalize()
    return nc


_CACHED = {}


def _get_nc():
    if "nc" not in _CACHED:
        _CACHED["nc"] = _build()
    return _CACHED["nc"]


def kernel(x, y, edge_index, w1, b1, w2, b2):
    from concourse.bass_utils import run_bass_kernel_spmd

    x = np.asarray(x, np.float32)
    y = np.asarray(y, np.float32)
    edge_index = np.asarray(edge_index, np.int32)
    w1 = np.asarray(w1, np.float32)
    b1 = np.asarray(b1, np.float32)
    w2 = np.asarray(w2, np.float32)
    b2 = np.asarray(b2, np.float32)

    src = edge_index[0].astype(np.int64)
    dst = edge_index[1].astype(np.int64)
    shard = dst // RANGE
    order = np.argsort(shard, kind="stable")
    counts = np.bincount(shard, minlength=NCORES)
    assert counts.max() <= EC, counts.max()
    starts = np.concatenate([[0], np.cumsum(counts)])

    # shared (replicated) inputs
    xT1 = np.zeros((W + 1, NPAD), np.float16)
    xT1[:W, :N] = x.T.astype(np.float16)
    xT1[W, :] = 1.0
    w1catb = np.zeros((W + 1, 2 * H), np.float16)
    w1catb[:W, :H] = w1[:W].astype(np.float16)
    w1catb[:W, H:] = w1[W:].astype(np.float16)
    w1catb[W, :H] = b1.astype(np.float16)
    y_pad = np.zeros(NPAD, np.float32)
    y_pad[:N] = y
    w2c = w2.astype(np.float16)
    b2c = np.full((P, 1), b2[0], np.float32)

    def wrap16(v):
        a = v.reshape(-1, 16).T  # [16, EC/16]
        return np.tile(a, (8, 1)).astype(np.int16)

    def pm(v, dtype):  # partition-major [128, C]: edge j -> [j%128, j//128]
        return np.ascontiguousarray(v.reshape(C, P).T).astype(dtype)

    in_maps = []
    eids_all = []
    for c in range(NCORES):
        eids = order[starts[c]:starts[c + 1]]
        eids_all.append(eids)
        n_e = len(eids)
        # reorder within shard: every 512-edge gather block must END with an
        # edge whose src and dst are both >= REBASE (ucode trims trailing
        # negative rebased indices)
        eid = eids.copy()
        n_e = len(eid)
        sfull = src[eid]
        dfull = dst[eid]
        okmask = (sfull >= REBASE)
        nblk = (n_e + 511) // 512
        for bi in range(nblk):
            b0, b1 = bi * 512, min((bi + 1) * 512, n_e)
            last = b1 - 1
            if okmask[last]:
                continue
            cand = b0 + int(np.argmax(okmask[b0:b1]))
            assert okmask[cand], f"no swap candidate in block {bi}"
            eid[last], eid[cand] = eid[cand], eid[last]
            okmask[last], okmask[cand] = okmask[cand], okmask[last]
        eids_all[-1] = eid
        sg = np.full(EC, REBASE, np.int64)
        dg = np.full(EC, REBASE, np.int64)
        dl = np.full(EC, NBINS - 1, np.int64)
        sg[:n_e] = src[eid]
        dg[:n_e] = dst[eid]
        dl[:n_e] = dst[eid] - c * RANGE
        xT1b = np.zeros((W + 1, NPAD), np.float16)
        rot = np.roll(np.arange(N), -c * RANGE)[:NPAD - 48]
        xT1b[:W, :N] = x.T[:, rot].astype(np.float16)
        xT1b[W, :] = 1.0
        in_maps.append({
            "xT1": xT1, "xT1b": xT1b, "w1catb": w1catb, "y_pad": y_pad,
            "w2c": w2c, "b2c": b2c,
            "srci16": wrap16(sg - REBASE),
            "dloci16": wrap16(dl),
            "lo16": pm(dl % P, np.float16),
            "hi16": pm(dl // P, np.float16),
        })

    nc = _get_nc()
    res = run_bass_kernel_spmd(nc, in_maps, core_ids=list(range(NCORES)))
    _CACHED["last_res"] = res

    y_hat = np.empty(N, np.float32)
    alpha = np.empty(E, np.float32)
    for c in range(NCORES):
        r = res.results[c]
        y_hat[c * RANGE:(c + 1) * RANGE] = r["yhat_o"].T.ravel()[:RANGE]
        n_e = len(eids_all[c])
        alpha[eids_all[c]] = r["alpha_o"].T.ravel()[:n_e]
    return y_hat, alpha[:, None]
